# revision 26
# baseline (speedup 1.0000x reference)
"""Trainium2 Bass kernel: BoxSeg DynamicMaskHead compute_pairwise_term.

For each instance n and each of the 8 non-center taps (dy, dx) of a 3x3
dilation-2 stencil:

    out[n, t, h, w] = sp(x[h,w]) + sp(x[h+dy,w+dx]) - sp(x[h,w] + x[h+dy,w+dx])

with sp = softplus (identical to the reference's
-logaddexp(log_fg + log_fg_tap, log_bg + log_bg_tap)), and 0 where the tap
falls outside the image.  No Softplus activation table exists in this build,
so sp is computed as  E = exp(x),  L = ln(E + 1)  and the tap term as
ln(1 + E_c * E_y)  (the product on DVE avoids a second per-tap Exp).

Mirror symmetry  out[(dy,dx)][h,w] == out[(-dy,-dx)][h+dy,w+dx]  means only
4 of the 8 tap fields are computed; each is DMA'd to two output locations.
Row shifts (dy) are materialized by DMA-loading a second row-shifted copy of
the input tile; column shifts (dx) are free-dim AP offsets.  The per-tap sum
L_c + L_y runs on the TensorEngine (identity matmul, accumulating PSUM), the
final combine  (ln_tap * -1) + Lsum  is a single fused DVE op.

Out-of-bounds strips are never written: run_bass_kernel_spmd pre-zeros
ExternalOutput buffers (np.zeros native / donated zero buffers under PJRT).

Sharding: data-parallel over N=64 -> 8 instances per core on 8 NeuronCores.
Self-contained: shapes hardcoded.
"""

import os

import numpy as np

N_CORES = 8
N_FULL = 64
N_PER = N_FULL // N_CORES  # 8 instances per core
H = W = 256
G = 2  # instances processed per block-iteration (batches DMA descriptors)

# Tap order matches F.unfold row-major (i, j) order with center removed,
# offsets (dy, dx) = ((i-1)*2, (j-1)*2).
TAPS = [(-2, -2), (-2, 0), (-2, 2), (0, -2), (0, 2), (2, -2), (2, 0), (2, 2)]
NT = len(TAPS)
# Computed quarters: (tap_idx, dy, dx, mirror_tap_idx)
QUARTERS = [
    (0, -2, -2, 7),  # q0: (-2,-2), mirror (2,2)
    (1, -2, 0, 6),   # q1: (-2, 0), mirror (2,0)
    (4, 0, 2, 3),    # q2: ( 0, 2), mirror (0,-2)
    (2, -2, 2, 5),   # q3: (-2, 2), mirror (2,-2)
]

_CACHE = {}


def _force_combined_act_table():
    """Make the table-load inserter see only the one set containing both Exp
    and Ln (all other sets emptied, positions preserved so act_func_set_id
    still indexes the real act_info.json).  Without this the inserter
    alternates between the exp- and ln-anchored sets: one 1.28us
    ACT_TABLE_LOAD per Exp<->Ln transition, which dominated the runtime."""
    import concourse.bacc as bacc
    import concourse.hw_specs as hw_specs
    import concourse.mybir as mybir

    real = dict(hw_specs.get_activation_tables("gen3"))
    target = None
    for name, fns in real.items():
        if (
            mybir.ActivationFunctionType.Exp in fns
            and mybir.ActivationFunctionType.Ln in fns
        ):
            target = name
            break
    assert target is not None, "no act table set with both Exp and Ln"
    patched = {
        name: (fns if name == target else set()) for name, fns in real.items()
    }
    bacc.get_activation_tables = lambda arch: patched
    hw_specs.get_activation_tables = lambda arch: patched


def _build_program():
    import concourse.bacc as bacc
    import concourse.mybir as mybir
    from concourse import tile

    if not os.environ.get("KERNEL_NO_ACT_PATCH"):
        _force_combined_act_table()

    f32 = mybir.dt.float32
    EXP = mybir.ActivationFunctionType.Exp
    LN = mybir.ActivationFunctionType.Ln
    ADD = mybir.AluOpType.add
    MULT = mybir.AluOpType.mult

    def mk(base, dims, off=0):
        """Rebuild the free dims of an AP: keep base's partition dim (ap[0]),
        replace the rest with `dims` ([step, count] in elements), and advance
        the offset by `off` elements."""
        c = base.copy()
        c.ap = mybir.VecI64Pair([list(c.ap[0])] + [list(d) for d in dims])
        c.offset = c.offset + off
        return c

    def mkd(base, dims, off=0):
        """Same for DRAM APs (no partition dim to preserve)."""
        c = base.copy()
        c.ap = mybir.VecI64Pair([list(d) for d in dims])
        c.offset = c.offset + off
        return c

    nc = bacc.Bacc(
        "TRN2",
        target_bir_lowering=False,
        debug=False,
        enable_asserts=False,
        num_devices=N_CORES,
    )
    x = nc.dram_tensor("x", [N_PER, H, W], f32, kind="ExternalInput").ap()
    out = nc.dram_tensor("out", [N_PER, NT, H, W], f32, kind="ExternalOutput").ap()
    eye = nc.dram_tensor("eye", [128, 128], f32, kind="ExternalInput").ap()

    # element strides in DRAM
    XN, XR = H * W, W                      # x[n, r, c]
    ON, OT, OR = NT * H * W, H * W, W      # out[n, t, r, c]

    # tile free-dim layouts (f32 elements per partition)
    # X/E/L: [G, 2, 260]  (k=0: rows r0+p, k=1: rows r0+p-2; col c = w+2)
    def xoff(g, k, c):
        return g * 520 + k * 260 + c

    XF = G * 520
    # P/ln/o: [G, 4, 256] quarters as in QUARTERS
    PF = G * 1024

    with tile.TileContext(nc) as tc:
        with (
            tc.tile_pool(name="cst", bufs=1) as cst,
            tc.tile_pool(name="io", bufs=3) as iop,
            tc.tile_pool(name="wk", bufs=3) as wp,
            tc.tile_pool(name="ps", bufs=2, space="PSUM") as psp,
        ):
            eyet = cst.tile([128, 128], f32)
            nc.sync.dma_start(out=eyet[:, :], in_=eye[:, :])

            for ng in range(N_PER // G):
                n0 = ng * G
                for blk in range(2):
                    r0 = blk * 128
                    X = iop.tile([128, XF], f32, tag="X")
                    # zero the 2-col halos at both edges of every (g, k) row
                    # window: cols {0,1} and {258,259}
                    nc.vector.memset(
                        mk(X[:, 0:1], [[260, 2 * G], [258, 2], [1, 2]]), 0.0
                    )
                    # k=0: full 128 rows at r0
                    nc.sync.dma_start(
                        out=mk(X[:, 0:1], [[520, G], [1, 256]], xoff(0, 0, 2)),
                        in_=mkd(x[0, 0:128, :], [[XR, 128], [XN, G], [1, 256]],
                                n0 * XN + r0 * XR),
                    )
                    # k=1 rows r0+p-2: partitions 2..128 <- rows r0..r0+126
                    nc.sync.dma_start(
                        out=mk(X[2:128, 0:1], [[520, G], [1, 256]], xoff(0, 1, 2)),
                        in_=mkd(x[0, 0:126, :], [[XR, 126], [XN, G], [1, 256]],
                                n0 * XN + r0 * XR),
                    )
                    # k=1 partitions 0..2 <- rows r0-2..r0 (r0=0: dummy rows
                    # 0..2, finite values feeding only discarded outputs)
                    nc.scalar.dma_start(
                        out=mk(X[0:2, 0:1], [[520, G], [1, 256]], xoff(0, 1, 2)),
                        in_=mkd(x[0, 0:2, :], [[XR, 2], [XN, G], [1, 256]],
                                n0 * XN + max(0, r0 - 2) * XR),
                    )

                    E = iop.tile([128, XF], f32, tag="E")
                    nc.scalar.activation(E[:, :], X[:, :], EXP)
                    L = iop.tile([128, XF], f32, tag="L")
                    nc.scalar.activation(L[:, :], E[:, :], LN, bias=1.0)

                    # P = E_c * E_y for the 4 quarters, all G instances
                    P = wp.tile([128, PF], f32, tag="P")
                    # pair A (q0,q1): E_y regions at (k=1, c={0,2})
                    nc.vector.tensor_mul(
                        out=mk(P[:, 0:1], [[1024, G], [256, 2], [1, 256]]),
                        in0=mk(E[:, 0:1], [[520, G], [0, 2], [1, 256]], xoff(0, 0, 2)),
                        in1=mk(E[:, 0:1], [[520, G], [2, 2], [1, 256]], xoff(0, 1, 0)),
                    )
                    # pair B (q2,q3): E_y regions at (k={0,1}, c=4)
                    nc.vector.tensor_mul(
                        out=mk(P[:, 0:1], [[1024, G], [256, 2], [1, 256]],
                               2 * 256),
                        in0=mk(E[:, 0:1], [[520, G], [0, 2], [1, 256]], xoff(0, 0, 2)),
                        in1=mk(E[:, 0:1], [[520, G], [260, 2], [1, 256]],
                               xoff(0, 0, 4)),
                    )

                    # Lsum via identity matmul, 2 banks per instance
                    ps = psp.tile([128, PF], f32, tag="ps")
                    for g in range(G):
                        b0 = g * 1024
                        # bank (g,0): q0,q1 shift part then center part
                        nc.tensor.matmul(
                            ps[:, b0 : b0 + 512], eyet[:, :],
                            mk(L[:, 0:1], [[2, 2], [1, 256]], xoff(g, 1, 0)),
                            start=True, stop=False,
                        )
                        nc.tensor.matmul(
                            ps[:, b0 : b0 + 512], eyet[:, :],
                            mk(L[:, 0:1], [[0, 2], [1, 256]], xoff(g, 0, 2)),
                            start=False, stop=True,
                        )
                        # bank (g,1): q2,q3 shift part then center part
                        nc.tensor.matmul(
                            ps[:, b0 + 512 : b0 + 1024], eyet[:, :],
                            mk(L[:, 0:1], [[260, 2], [1, 256]], xoff(g, 0, 4)),
                            start=True, stop=False,
                        )
                        nc.tensor.matmul(
                            ps[:, b0 + 512 : b0 + 1024], eyet[:, :],
                            mk(L[:, 0:1], [[0, 2], [1, 256]], xoff(g, 0, 2)),
                            start=False, stop=True,
                        )

                    ln_t = wp.tile([128, PF], f32, tag="ln")
                    nc.scalar.activation(ln_t[:, :], P[:, :], LN, bias=1.0)

                    o = wp.tile([128, PF], f32, tag="o")
                    nc.vector.scalar_tensor_tensor(
                        out=o[:, :], in0=ln_t[:, :], scalar=-1.0, in1=ps[:, :],
                        op0=MULT, op1=ADD,
                    )

                    # write each quarter twice (tap + mirror), valid regions
                    # only; G instances per DMA
                    for qi, (t_idx, dy, dx, tm_idx) in enumerate(QUARTERS):
                        eng = nc.sync if qi % 2 == 0 else nc.scalar
                        # direct tap T = (dy, dx)
                        a = max(r0, -dy)
                        b = min(r0 + 128, 256)  # dy <= 0 here
                        c0, c1 = max(0, -dx), 256 - max(0, dx)
                        eng.dma_start(
                            out=mkd(out[0, 0, 0:1, 0:1],
                                    [[OR, b - a], [ON, G], [1, c1 - c0]],
                                    n0 * ON + t_idx * OT + a * OR + c0),
                            in_=mk(o[a - r0 : b - r0, 0:1], [[1024, G], [1, c1 - c0]],
                                   qi * 256 + c0),
                        )
                        # mirror tap T' = (-dy, -dx): out_T'[h+dy, w+dx] = F[h, w]
                        a2 = max(r0 + dy, 0)
                        b2 = min(r0 + 128 + dy, 256 + dy)
                        c02, c12 = max(0, dx), 256 - max(0, -dx)
                        eng.dma_start(
                            out=mkd(out[0, 0, 0:1, 0:1],
                                    [[OR, b2 - a2], [ON, G], [1, c12 - c02]],
                                    n0 * ON + tm_idx * OT + a2 * OR + c02),
                            in_=mk(o[a2 - dy - r0 : b2 - dy - r0, 0:1],
                                   [[1024, G], [1, c12 - c02]],
                                   qi * 256 + c02 - dx),
                        )
    nc.compile()
    return nc


def _get_program():
    if "nc" not in _CACHE:
        _CACHE["nc"] = _build_program()
    return _CACHE["nc"]


def kernel(mask_logits, pairwise_size=3, pairwise_dilation=2, **_unused):
    assert int(pairwise_size) == 3 and int(pairwise_dilation) == 2
    from concourse.bass_utils import run_bass_kernel_spmd

    xf = np.ascontiguousarray(
        np.asarray(mask_logits, dtype=np.float32).reshape(N_FULL, H, W)
    )
    nc = _get_program()
    eye = np.eye(128, dtype=np.float32)
    in_maps = [
        {"x": np.ascontiguousarray(xf[c * N_PER : (c + 1) * N_PER]), "eye": eye}
        for c in range(N_CORES)
    ]
    res = run_bass_kernel_spmd(nc, in_maps, core_ids=list(range(N_CORES)))
    return np.concatenate([res.results[c]["out"] for c in range(N_CORES)], axis=0)
